# revision 4
# baseline (speedup 1.0000x reference)
# Causal multi-head attention (B=4, L=2048, H=16, E=64, fp32) on 8 TRN2
# NeuronCores. Sharding: the 64 (b,h) pairs split 8 per core; each core
# computes its pairs fully independently (data parallel on B, tensor
# parallel on H).
#
# Per-core algorithm (per pair of heads, packed into the two 64-row
# halves of the PE array):
#   S^T[s,l] = K^T . Q  chunks in PSUM (causal-skipped, bf16 matmul)
#   P^T = exp(S^T/8)    on ScalarE, PSUM -> SBUF bf16 (the bottleneck)
#   diagonal tiles masked by an upper-triangular 0/1 multiply on VectorE
#   O[l,:64] and rowsum (ones-augmented V) accumulate in PSUM via PE
#   out = O / rowsum    on VectorE, then DMA to HBM
# L is processed in 4 phases of 512 columns to bound SBUF usage.

import sys

import numpy as np

try:
    import concourse.bass as bass  # noqa: F401
except ImportError:
    sys.path.insert(0, "/opt/trn_rl_repo")

B, L, H, E = 4, 2048, 16, 64
NCORES = 8
BH = B * H                  # 64 (b,h) pairs
BH_PER_CORE = BH // NCORES  # 8
NPAIRS = BH_PER_CORE // 2   # 4 packed pairs per core
NLT = L // 128              # 16 l-tiles
NPH = 4                     # phases over l
PHL = L // NPH              # 512 l-cols per phase
VW = 66                     # V columns + ones + pad (4B-aligned bf16 blocks)

_CACHE = {}


def _phase_widths(ph):
    """Per-s-tile chunk widths within phase ph. Returns list of
    (st, lstart, w) for every s-tile contributing to this phase."""
    lo, hi = ph * PHL, (ph + 1) * PHL
    out = []
    for st in range(4 * ph + 4):
        lstart = max(st * 128, lo)
        w = hi - lstart
        assert w > 0
        out.append((st, lstart, w))
    return out


def _build_program():
    from contextlib import ExitStack

    import concourse.bass as bass
    import concourse.mybir as mybir
    import concourse.tile as tile
    from concourse import bacc
    from concourse.masks import make_identity, make_upper_triangular

    f32 = mybir.dt.float32
    bf16 = mybir.dt.bfloat16

    nc = bacc.Bacc(
        "TRN2",
        target_bir_lowering=False,
        debug=False,
        enable_asserts=False,
        num_devices=NCORES,
    )
    q_d = nc.dram_tensor("q", [BH_PER_CORE, L, E], f32, kind="ExternalInput").ap()
    k_d = nc.dram_tensor("k", [BH_PER_CORE, L, E], f32, kind="ExternalInput").ap()
    v_d = nc.dram_tensor("v", [BH_PER_CORE, L, E], f32, kind="ExternalInput").ap()
    o_d = nc.dram_tensor("o", [BH_PER_CORE, L, E], f32, kind="ExternalOutput").ap()

    with tile.TileContext(nc) as tc, ExitStack() as ctx:
        consts = ctx.enter_context(tc.tile_pool(name="consts", bufs=1))
        stage = ctx.enter_context(tc.tile_pool(name="stage", bufs=2))
        cast = ctx.enter_context(tc.tile_pool(name="cast", bufs=2))
        qkt = ctx.enter_context(tc.tile_pool(name="qkt", bufs=2))
        ptp = ctx.enter_context(tc.tile_pool(name="pt", bufs=2))
        outp = ctx.enter_context(tc.tile_pool(name="outp", bufs=2))
        recp = ctx.enter_context(tc.tile_pool(name="recp", bufs=4))
        tpsum = ctx.enter_context(tc.tile_pool(name="tpsum", bufs=2, space="PSUM"))
        spsum = ctx.enter_context(tc.tile_pool(name="spsum", bufs=2, space="PSUM"))
        opsum = ctx.enter_context(tc.tile_pool(name="opsum", bufs=2, space="PSUM"))

        ident = consts.tile([128, 128], bf16)
        make_identity(nc, ident)
        # mask01[s, j] = 1.0 where s <= j else 0.0 (valid causal region of a
        # diagonal tile of P^T)
        mask01 = consts.tile([128, 128], bf16)
        make_upper_triangular(nc, mask01, val=1.0, diag=True)

        scale = 1.0 / float(np.sqrt(E))

        for p in range(NPAIRS):
            # ---- load fp32 inputs for the two heads of this pair ----
            qf = stage.tile([128, 2, NLT, E], f32, tag="qf")
            kf = stage.tile([128, 2, NLT, E], f32, tag="kf")
            vf = stage.tile([128, 2, NLT, E], f32, tag="vf")
            for j in range(2):
                bh = 2 * p + j
                for d_src, d_dst in ((q_d, qf), (k_d, kf), (v_d, vf)):
                    nc.sync.dma_start(
                        out=d_dst[:, j],
                        in_=d_src[bh].rearrange("(t pp) e -> pp t e", pp=128),
                    )

            # ---- cast to bf16 ----
            qb = cast.tile([128, 2, NLT, E], bf16, tag="qb")
            kb = cast.tile([128, 2, NLT, E], bf16, tag="kb")
            nc.vector.tensor_copy(qb, qf)
            nc.vector.tensor_copy(kb, kf)
            # V with a ones column at 64 (rowsum trick) and zero pad at 65
            vaug = cast.tile([128, 2, NLT, VW], bf16, tag="vaug")
            nc.vector.tensor_copy(vaug[:, :, :, 0:E], vf)
            nc.vector.memset(vaug[:, :, :, E : E + 1], 1.0)
            nc.vector.memset(vaug[:, :, :, E + 1 : VW], 0.0)

            # ---- transpose Q,K -> [e, l] layout (two heads in row halves) --
            qt = qkt.tile([128, L], bf16, tag="qt")
            kt = qkt.tile([128, L], bf16, tag="kt")
            for src, dst in ((qb, qt), (kb, kt)):
                for c in range(4):
                    tp = tpsum.tile([128, 512], bf16)
                    for j in range(2):
                        for i in range(4):
                            lt = 4 * c + i
                            nc.tensor.transpose(
                                out=tp[64 * j : 64 * (j + 1), 128 * i : 128 * (i + 1)],
                                in_=src[:, j, lt, :],
                                identity=ident,
                            )
                    nc.vector.tensor_copy(dst[:, 512 * c : 512 * (c + 1)], tp)

            ob = outp.tile([128, 2, NLT, E], f32, tag="ob")

            # ---- phases over l ----
            for ph in range(NPH):
                chunks = _phase_widths(ph)
                W = sum(w for _, _, w in chunks)
                ptq = ptp.tile([128, 2 * W], bf16)
                off = 0
                offs = {}
                for st, lstart, w in chunks:
                    s0 = st * 128
                    sp = spsum.tile([128, 1024], f32)
                    for j in range(2):
                        nc.tensor.matmul(
                            out=sp[:, 512 * j : 512 * j + w],
                            lhsT=kt[64 * j : 64 * (j + 1), s0 : s0 + 128],
                            rhs=qt[64 * j : 64 * (j + 1), lstart : lstart + w],
                            start=True,
                            stop=True,
                        )
                    sp_v = sp.rearrange("pp (j c) -> pp j c", j=2)[:, :, 0:w]
                    pt_v = ptq[:, 2 * off : 2 * (off + w)].rearrange(
                        "pp (j c) -> pp j c", j=2
                    )
                    nc.scalar.activation(
                        pt_v, sp_v, mybir.ActivationFunctionType.Exp, scale=scale
                    )
                    if lstart == s0:
                        # diagonal tile: zero the s > l half
                        for j in range(2):
                            dslice = ptq[:, 2 * off + j * w : 2 * off + j * w + 128]
                            nc.vector.tensor_mul(dslice, dslice, mask01)
                    offs[st] = (off, w)
                    off += w

                # ---- AV accumulation + normalize for this phase ----
                for j in range(2):
                    op = opsum.tile([128, 4, VW], f32)
                    for i in range(4):
                        lt = 4 * ph + i
                        for st in range(lt + 1):
                            o_, w_ = offs[st]
                            so = 2 * o_ + j * w_ + (lt * 128 - max(st * 128, ph * PHL))
                            nc.tensor.matmul(
                                out=op[:, i, :],
                                lhsT=ptq[:, so : so + 128],
                                rhs=vaug[:, j, st, :],
                                start=(st == 0),
                                stop=(st == lt),
                            )
                    rc = recp.tile([128, 4], f32)
                    nc.vector.reciprocal(rc, op[:, :, E])
                    rc_ap = rc[:]
                    rc_b = bass.AP(
                        tensor=rc_ap.tensor,
                        offset=rc_ap.offset,
                        ap=list(rc_ap.ap) + [[0, E]],
                    )
                    nc.vector.tensor_mul(
                        ob[:, j, 4 * ph : 4 * ph + 4, :], op[:, :, 0:E], rc_b
                    )

            # ---- store ----
            for j in range(2):
                nc.sync.dma_start(
                    out=o_d[2 * p + j].rearrange("(t pp) e -> pp t e", pp=128),
                    in_=ob[:, j],
                )

    nc.compile()
    return nc


def _get_program():
    if "nc" not in _CACHE:
        _CACHE["nc"] = _build_program()
    return _CACHE["nc"]


def kernel(queries, values=None, keys=None, **kw):
    # accept keyword order-agnostic
    if values is None or keys is None:
        raise TypeError("kernel expects queries, keys, values")
    from concourse.bass_utils import run_bass_kernel_spmd

    q = np.ascontiguousarray(np.asarray(queries, dtype=np.float32))
    k = np.ascontiguousarray(np.asarray(keys, dtype=np.float32))
    v = np.ascontiguousarray(np.asarray(values, dtype=np.float32))
    assert q.shape == (B, L, H, E), q.shape

    # [B, L, H, E] -> [BH, L, E]
    def shard(x):
        return np.ascontiguousarray(x.transpose(0, 2, 1, 3).reshape(BH, L, E))

    qs, ks, vs = shard(q), shard(k), shard(v)
    in_maps = [
        {
            "q": qs[c * BH_PER_CORE : (c + 1) * BH_PER_CORE],
            "k": ks[c * BH_PER_CORE : (c + 1) * BH_PER_CORE],
            "v": vs[c * BH_PER_CORE : (c + 1) * BH_PER_CORE],
        }
        for c in range(NCORES)
    ]
    nc = _get_program()
    res = run_bass_kernel_spmd(nc, in_maps, core_ids=list(range(NCORES)))
    o = np.concatenate([res.results[c]["o"] for c in range(NCORES)], axis=0)
    # [BH, L, E] -> [B, L, H, E]
    return np.ascontiguousarray(
        o.reshape(B, H, L, E).transpose(0, 2, 1, 3)
    ).astype(np.float32)


if __name__ == "__main__":
    rng = np.random.default_rng(0)
    qq = rng.standard_normal((B, L, H, E), dtype=np.float32)
    kk = rng.standard_normal((B, L, H, E), dtype=np.float32)
    vv = rng.standard_normal((B, L, H, E), dtype=np.float32)
    out = kernel(queries=qq, keys=kk, values=vv)
    print(out.shape, out.dtype)
